# revision 13
# baseline (speedup 1.0000x reference)
"""Trainium2 Bass kernel for the ActionDecoder problem.

Strategy
--------
1. Raggedness: grasps for points with confidence <= mask_thresh are zeroed by
   the reference, so their MLP work is skipped entirely.  The host compacts
   each (b, q) row set to the selected indices (gathering mask_feats / xyz
   columns), pads to a runtime capacity C (multiple of 512), and scatters the
   kernel's compacted output back into the zero-initialized full tensor.
   grasp_confidence = confidence * mask is pure input masking done on host.
2. Sharding: the 16 (b, q) pairs are split across 8 cores (2 pairs each);
   every core carries the full (small) MLP weights.
3. Precision: matmuls run as float32r (PE full rate, e10m11 truncation) in an
   error-free 3-term hi/lo split: x = xh + xl, W = Wh + Wl exactly at m11,
   and x@W = xh@Wh + xh@Wl + xl@Wh accumulated in one fp32 PSUM (the dropped
   xl@Wl term is ~2^-24 relative) -> fp32-grade results at 3 cycles/row.
4. Layer 1 input is concat(feat, emb):  inp @ W1 = feat @ W1[:256] +
   emb @ W1[256:].  The emb part (+ b1) is a tiny exact-fp32 matmul per
   (mlp, pair); the feat part accumulates into the same PSUM chunk and goes
   through a fused PSUM->relu(+bias) into h1, split hi/lo for layer 2.
5. Head outputs ([3],[3],[10]) are PE-transposed into a rows-on-partitions
   [128, rows, 16] layout where the epilogue (normalize, Gram-Schmidt, cross,
   argmax bin lookup, grasp assembly) runs as batched DVE/ACT ops per pair.
"""

import numpy as np

import concourse.bass as bass
import concourse.mybir as mybir
from concourse import bacc
from concourse.tile import TileContext
from concourse.bass_utils import run_bass_kernel_spmd
from concourse.masks import make_identity

B, Q, N = 2, 8, 8192
NCORES = 8
PAIRS = B * Q             # 16 (b,q) pairs, 2 per core
PPC = PAIRS // NCORES     # pairs per core
MASK = 256
HID = 512
NB = 10
T = 512                   # row-chunk (matmul moving dim)
GRIP = 0.1034

F32 = mybir.dt.float32
F32R = mybir.dt.float32r
AX = mybir.AluOpType
ACTF = mybir.ActivationFunctionType
AXL = mybir.AxisListType

HEAD_D = [3, 3, 10]       # contact, approach, offset
HEAD_OFF = [0, 3, 6]      # column offsets in the [128, rows, 16] layout

_CACHE: dict = {}

LAST_RESULT = None        # BassKernelResults of the most recent run (for profiling)


def _t22(x):
    """Truncate fp32 to fp22=e10m11 (11-bit mantissa) - matches the PE's
    float32r load, so device truncation of these values is the identity."""
    xi = np.ascontiguousarray(x, np.float32).view(np.uint32)
    return (xi & np.uint32(0xFFFFF000)).view(np.float32)


def _split22(x):
    """Exact hi/lo fp22 split of fp32 data: x == hi + lo bitwise."""
    hi = _t22(x)
    lo = np.asarray(x, np.float32) - hi
    return hi, lo


def _build(C):
    """Build the kernel for per-pair row capacity C (multiple of 512)."""
    NCH = C // T          # chunks per pair
    SUB = C // 128        # 128-row groups per pair
    M = PPC * SUB         # row-groups per core across its pairs

    nc = bacc.Bacc(None, target_bir_lowering=False, debug=False)

    # ---- per-core DRAM parameters -------------------------------------
    # compacted, per-pair tensors (leading dim = local pair index)
    mf = nc.declare_dram_parameter("mf", [PPC, 2, 128, C], F32, isOutput=False)
    emt = nc.declare_dram_parameter("emt", [2, 128, PPC], F32, isOutput=False)
    xyzs = nc.declare_dram_parameter("xyzs", [PPC, 128, SUB, 3], F32, isOutput=False)

    w1fp, w2p, w3p, w1e, b1, b2 = [], [], [], [], [], []
    for i, d in enumerate(HEAD_D):
        w1fp.append([nc.declare_dram_parameter(f"w1f{s}{i}", [2, 128, HID], F32R, isOutput=False)
                     for s in "hl"])
        w2p.append([nc.declare_dram_parameter(f"w2{s}{i}", [4, 128, HID], F32R, isOutput=False)
                    for s in "hl"])
        w3p.append([nc.declare_dram_parameter(f"w3{s}{i}", [4, 128, d], F32R, isOutput=False)
                    for s in "hl"])
        w1e.append(nc.declare_dram_parameter(f"w1e{i}", [2, 128, HID], F32, isOutput=False))
        b1.append(nc.declare_dram_parameter(f"b1{i}", [4, 128], F32, isOutput=False))
        b2.append(nc.declare_dram_parameter(f"b2{i}", [4, 128], F32, isOutput=False))
    b3row = nc.declare_dram_parameter("b3row", [128, 16], F32, isOutput=False)
    ovals = nc.declare_dram_parameter("ovals", [128, NB], F32, isOutput=False)
    desc = nc.declare_dram_parameter("desc", [128, NB], F32, isOutput=False)

    go = nc.declare_dram_parameter("go", [PPC, SUB, 128, 4, 4], F32, isOutput=True)

    with TileContext(nc) as tc:
        with (
            tc.tile_pool(name="const", bufs=1) as cpool,
            tc.tile_pool(name="wts", bufs=2) as wtpool,
            tc.tile_pool(name="io", bufs=2) as iopool,
            tc.tile_pool(name="work", bufs=2) as wpool,
            tc.tile_pool(name="epi", bufs=1) as epool,
            tc.tile_pool(name="ps", bufs=2, space="PSUM") as pspool,
            tc.tile_pool(name="pshd", bufs=2, space="PSUM") as hdpool,
        ):
            # ---- constants into SBUF -----------------------------------
            ident = cpool.tile([128, 128], F32)
            make_identity(nc, ident)

            w1e_sb, b1_sb, b2_sb, w3_sb = [], [], [], []
            for i, d in enumerate(HEAD_D):
                t2 = cpool.tile([128, 2, HID], F32, name=f"w1e_sb{i}")
                nc.sync.dma_start(t2[:], w1e[i].ap().rearrange("k p h -> p k h"))
                w1e_sb.append(t2)
                t5 = cpool.tile([128, 4], F32, name=f"b1_sb{i}")
                nc.sync.dma_start(t5[:], b1[i].ap().rearrange("m p -> p m"))
                b1_sb.append(t5)
                t6 = cpool.tile([128, 4], F32, name=f"b2_sb{i}")
                nc.sync.dma_start(t6[:], b2[i].ap().rearrange("m p -> p m"))
                b2_sb.append(t6)
                pair = []
                for s in range(2):
                    t7 = cpool.tile([128, 4, d], F32R, name=f"w3_sb{i}_{s}")
                    nc.sync.dma_start(t7[:], w3p[i][s].ap().rearrange("k p h -> p k h"))
                    pair.append(t7)
                w3_sb.append(pair)
            b3row_sb = cpool.tile([128, 16], F32)
            nc.sync.dma_start(b3row_sb[:], b3row.ap())
            ovals_sb = cpool.tile([128, NB], F32)
            nc.sync.dma_start(ovals_sb[:], ovals.ap())
            desc_sb = cpool.tile([128, NB], F32)
            nc.sync.dma_start(desc_sb[:], desc.ap())

            # ---- E1 + b1 (exact fp32 matmul, tiny): be1[i] [128, 4, PPC]
            emt_sb = cpool.tile([128, 2, PPC], F32)
            nc.sync.dma_start(emt_sb[:], emt.ap().rearrange("k p q -> p k q"))
            be1 = []
            for i in range(3):
                bt = cpool.tile([128, 4, PPC], F32, name=f"be1_{i}")
                for m in range(4):
                    ps = pspool.tile([128, T], F32, tag="mm")
                    for k in range(2):
                        nc.tensor.matmul(
                            ps[:, :PPC],
                            w1e_sb[i][:, k, m * 128:(m + 1) * 128],
                            emt_sb[:, k, :],
                            start=(k == 0), stop=(k == 1),
                        )
                    nc.vector.tensor_scalar(
                        bt[:, m, :], ps[:, :PPC], b1_sb[i][:, m:m + 1], None, AX.add,
                    )
                be1.append(bt)

            # ---- main loops --------------------------------------------
            out_rows = epool.tile([128, M, 16], F32, name="out_rows", tag="orows")
            xyz_sb = []
            for p_ in range(PPC):
                xt = iopool.tile([128, SUB, 3], F32, name=f"xyz_sb{p_}", tag="xyz", bufs=2)
                nc.sync.dma_start(xt[:], xyzs[p_])
                xyz_sb.append(xt)

            for p_ in range(PPC):
                for i in range(3):
                    d = HEAD_D[i]
                    off = HEAD_OFF[i]
                    w1f_t, w2_t = [], []
                    for s in range(2):
                        wt = wtpool.tile([128, 2, HID], F32R, name=f"w1f_t{s}",
                                         tag=f"w1f{s}", bufs=2)
                        nc.sync.dma_start(wt[:], w1fp[i][s].ap().rearrange("k p h -> p k h"))
                        w1f_t.append(wt)
                        wt2 = wtpool.tile([128, 4, HID], F32R, name=f"w2_t{s}",
                                          tag=f"w2{s}", bufs=2)
                        nc.sync.dma_start(wt2[:], w2p[i][s].ap().rearrange("k p h -> p k h"))
                        w2_t.append(wt2)
                    for ch in range(NCH):
                        cs = slice(ch * T, (ch + 1) * T)
                        mff_sb = iopool.tile([128, 2, T], F32, name="mff_sb",
                                             tag="mff", bufs=3)
                        nc.sync.dma_start(
                            mff_sb[:], mf[p_, :, :, cs].rearrange("k p n -> p k n"))
                        mfh_sb = iopool.tile([128, 2, T], F32R, name="mfh_sb",
                                             tag="mfh", bufs=3)
                        nc.vector.tensor_copy(mfh_sb[:], mff_sb[:])
                        mfl_sb = iopool.tile([128, 2, T], F32R, name="mfl_sb",
                                             tag="mfl", bufs=3)
                        nc.vector.tensor_tensor(
                            mfl_sb[:], mff_sb[:], mfh_sb[:].bitcast(F32), AX.subtract)
                        # layer 1 (feat part) fused into h1 = relu(psum + E1+b1)
                        h1f = wpool.tile([128, 4, T], F32, name="h1f", tag="h1f", bufs=1)
                        h1h = wpool.tile([128, 4, T], F32R, name="h1h", tag="h1h")
                        h1l = wpool.tile([128, 4, T], F32R, name="h1l", tag="h1l")
                        for m in range(4):
                            ms = slice(m * 128, (m + 1) * 128)
                            ps = pspool.tile([128, T], F32, tag="mm")
                            j = 0
                            for k in range(2):
                                for wa, xa in ((0, mfh_sb), (1, mfh_sb), (0, mfl_sb)):
                                    nc.tensor.matmul(
                                        ps[:], w1f_t[wa][:, k, ms], xa[:, k, slice(0, T)],
                                        start=(j == 0), stop=(j == 5),
                                    )
                                    j += 1
                            nc.scalar.activation(
                                h1f[:, m, :], ps[:], ACTF.Relu,
                                bias=be1[i][:, m, p_:p_ + 1],
                            )
                        nc.vector.tensor_copy(h1h[:], h1f[:])
                        nc.vector.tensor_tensor(
                            h1l[:], h1f[:], h1h[:].bitcast(F32), AX.subtract)

                        # layer 2: 3-term split matmul + relu
                        h2h = wpool.tile([128, 4, T], F32R, name="h2h", tag="h2h")
                        h2l = wpool.tile([128, 4, T], F32R, name="h2l", tag="h2l", bufs=1)
                        h2f = wpool.tile([128, 4, T], F32, name="h2f", tag="h2f", bufs=1)
                        for m in range(4):
                            ms = slice(m * 128, (m + 1) * 128)
                            ps = pspool.tile([128, T], F32, tag="mm")
                            j = 0
                            for k in range(4):
                                for wa, xa in ((0, h1h), (1, h1h), (0, h1l)):
                                    nc.tensor.matmul(
                                        ps[:], w2_t[wa][:, k, ms], xa[:, k, :],
                                        start=(j == 0), stop=(j == 11),
                                    )
                                    j += 1
                            nc.scalar.activation(
                                h2f[:, m, :], ps[:], ACTF.Relu,
                                bias=b2_sb[i][:, m:m + 1],
                            )
                        nc.vector.tensor_copy(h2h[:], h2f[:])
                        nc.vector.tensor_tensor(
                            h2l[:], h2f[:], h2h[:].bitcast(F32), AX.subtract)

                        # layer 3 head: [d, T] psum, 3-term split
                        hd = hdpool.tile([d, T], F32, tag="hd10")
                        j = 0
                        for k in range(4):
                            for wa, xa in ((0, h2h), (1, h2h), (0, h2l)):
                                nc.tensor.matmul(
                                    hd[:], w3_sb[i][wa][:, k, :], xa[:, k, :],
                                    start=(j == 0), stop=(j == 11),
                                )
                                j += 1
                        sc = wpool.tile([d, T], F32, name="sc", tag="sc10")
                        nc.vector.tensor_copy(sc[:], hd[:])

                        # transpose [d, T] -> [128, 4, d] rows-on-partitions
                        pt = pspool.tile([128, 4, 16], F32, tag="pt")
                        for c4 in range(T // 128):
                            nc.tensor.transpose(
                                pt[:, c4, :d],
                                sc[:, c4 * 128:(c4 + 1) * 128],
                                ident[:d, :d],
                            )
                        nc.vector.tensor_copy(
                            out_rows[:, p_ * SUB + ch * 4: p_ * SUB + ch * 4 + 4,
                                     off:off + d],
                            pt[:, :, :d],
                        )

            # ---- epilogue over all rows of this core's pairs -----------
            nc.vector.tensor_tensor(
                out_rows[:], out_rows[:],
                b3row_sb[:, None, :].to_broadcast([128, M, 16]), AX.add,
            )
            c_raw = out_rows[:, :, 0:3]
            a_raw = out_rows[:, :, 3:6]
            lg = out_rows[:, :, 6:16]

            sq = epool.tile([128, M, 3], F32, name="sq", tag="sq")
            s1 = epool.tile([128, M, 1], F32, name="s1", tag="s1")
            inv = epool.tile([128, M, 1], F32, name="inv", tag="inv")
            contact = epool.tile([128, M, 3], F32, name="contact", tag="contact")
            approach = epool.tile([128, M, 3], F32, name="approach", tag="approach")
            dotp = epool.tile([128, M, 1], F32, name="dotp", tag="dotp")
            tmp3 = epool.tile([128, M, 3], F32, name="tmp3", tag="tmp3")
            cr = epool.tile([128, M, 3], F32, name="cr", tag="cr")
            tr = epool.tile([128, M, 3], F32, name="tr", tag="tr")
            t10 = epool.tile([128, M, NB], F32, name="t10", tag="t10")
            vmax = epool.tile([128, M, 1], F32, name="vmax", tag="vmax")
            offs = epool.tile([128, M, 1], F32, name="offs", tag="offs")
            g4 = epool.tile([128, M, 4, 4], F32, name="g4", tag="g4")

            def _normalize(dst, src):
                nc.vector.tensor_tensor(sq[:], src, src, AX.mult)
                nc.vector.reduce_sum(s1[:], sq[:], axis=AXL.X)
                nc.scalar.activation(s1[:], s1[:], ACTF.Sqrt)
                nc.vector.tensor_scalar_max(s1[:], s1[:], 1e-12)
                nc.vector.reciprocal(inv[:], s1[:])
                nc.vector.tensor_tensor(
                    dst, src, inv[:].to_broadcast([128, M, 3]), AX.mult)

            _normalize(contact[:], c_raw)
            # Gram-Schmidt: approach = normalize(a - contact * <a, contact>)
            nc.vector.tensor_tensor(sq[:], a_raw, contact[:], AX.mult)
            nc.vector.reduce_sum(dotp[:], sq[:], axis=AXL.X)
            nc.vector.tensor_tensor(
                tmp3[:], contact[:], dotp[:].to_broadcast([128, M, 3]), AX.mult)
            nc.vector.tensor_tensor(tmp3[:], a_raw, tmp3[:], AX.subtract)
            _normalize(approach[:], tmp3[:])

            # cross(approach, contact)
            for j in range(3):
                j1, j2 = (j + 1) % 3, (j + 2) % 3
                nc.vector.tensor_tensor(
                    cr[:, :, j:j + 1], approach[:, :, j1:j1 + 1],
                    contact[:, :, j2:j2 + 1], AX.mult)
                nc.vector.tensor_tensor(
                    tmp3[:, :, j:j + 1], approach[:, :, j2:j2 + 1],
                    contact[:, :, j1:j1 + 1], AX.mult)
            nc.vector.tensor_tensor(cr[:], cr[:], tmp3[:], AX.subtract)

            # offsets = offset_vals[argmax(lg)] (first max wins)
            nc.vector.reduce_max(vmax[:], lg, axis=AXL.X)
            nc.vector.tensor_tensor(
                t10[:], lg, vmax[:].to_broadcast([128, M, NB]), AX.is_equal)
            nc.vector.tensor_tensor(
                t10[:], t10[:], desc_sb[:, None, :].to_broadcast([128, M, NB]),
                AX.mult)
            nc.vector.reduce_max(vmax[:], t10[:], axis=AXL.X)
            nc.vector.tensor_tensor(
                t10[:], t10[:], vmax[:].to_broadcast([128, M, NB]), AX.is_equal)
            nc.vector.tensor_tensor(
                t10[:], t10[:], ovals_sb[:, None, :].to_broadcast([128, M, NB]),
                AX.mult)
            nc.vector.reduce_sum(offs[:], t10[:], axis=AXL.X)
            nc.vector.tensor_scalar_mul(offs[:], offs[:], 0.5)

            # trans = xyz + contact * offs/2 - GRIP * approach
            nc.vector.tensor_tensor(
                tr[:], contact[:], offs[:].to_broadcast([128, M, 3]), AX.mult)
            nc.vector.tensor_scalar(
                tmp3[:], approach[:], -GRIP, None, AX.mult)
            nc.vector.tensor_tensor(tr[:], tr[:], tmp3[:], AX.add)
            for p_ in range(PPC):
                nc.vector.tensor_tensor(
                    tr[:, p_ * SUB:(p_ + 1) * SUB, :],
                    tr[:, p_ * SUB:(p_ + 1) * SUB, :], xyz_sb[p_][:], AX.add)

            # grasp assembly (selected rows all have mask == 1)
            nc.vector.tensor_copy(g4[:, :, 0:3, 0], contact[:])
            nc.vector.tensor_copy(g4[:, :, 0:3, 1], cr[:])
            nc.vector.tensor_copy(g4[:, :, 0:3, 2], approach[:])
            nc.vector.tensor_copy(g4[:, :, 0:3, 3], tr[:])
            nc.vector.memset(g4[:, :, 3, 0:3], 0.0)
            nc.vector.memset(g4[:, :, 3, 3], 1.0)

            nc.sync.dma_start(
                go.ap().rearrange("q s p i j -> p (q s) i j"), g4[:])

    nc.finalize()
    return nc


def _get_nc(C):
    key = f"nc_{C}"
    if key not in _CACHE:
        _CACHE[key] = _build(C)
    return _CACHE[key]


def kernel(xyz, mask_feats, confidence, embedding, mask_thresh,
           contact_params, approach_params, offset_params, offset_vals):
    global LAST_RESULT
    xyz = np.asarray(xyz, np.float32)
    mask_feats = np.asarray(mask_feats, np.float32)
    confidence = np.asarray(confidence, np.float32)
    embedding = np.asarray(embedding, np.float32)
    thr_v = float(np.asarray(mask_thresh))
    params = [
        [(np.asarray(w, np.float32), np.asarray(bb, np.float32)) for w, bb in p]
        for p in (contact_params, approach_params, offset_params)
    ]
    offset_vals = np.asarray(offset_vals, np.float32)

    # ---- compaction: selected indices per (b, q) -----------------------
    sels = [[np.nonzero(confidence[b, q] > thr_v)[0] for q in range(Q)]
            for b in range(B)]
    max_sel = max((len(s) for bs in sels for s in bs), default=1)
    C = max(T, ((max_sel + T - 1) // T) * T)

    # ---- shared (per-core identical) tensors ---------------------------
    shared = {}
    b3cat = np.zeros(16, np.float32)
    for i, p in enumerate(params):
        w1, b1v = p[0]
        w2v, b2v = p[1]
        w3v, b3v = p[2]
        d = HEAD_D[i]
        hi, lo = _split22(w1[:MASK].reshape(2, 128, HID))
        shared[f"w1fh{i}"], shared[f"w1fl{i}"] = hi, lo
        shared[f"w1e{i}"] = np.ascontiguousarray(w1[MASK:].reshape(2, 128, HID))
        hi, lo = _split22(w2v.reshape(4, 128, HID))
        shared[f"w2h{i}"], shared[f"w2l{i}"] = hi, lo
        hi, lo = _split22(w3v.reshape(4, 128, d))
        shared[f"w3h{i}"], shared[f"w3l{i}"] = hi, lo
        shared[f"b1{i}"] = np.ascontiguousarray(b1v.reshape(4, 128))
        shared[f"b2{i}"] = np.ascontiguousarray(b2v.reshape(4, 128))
        b3cat[HEAD_OFF[i]:HEAD_OFF[i] + d] = b3v
    shared["b3row"] = np.tile(b3cat, (128, 1))
    shared["ovals"] = np.tile(offset_vals, (128, 1))
    shared["desc"] = np.tile((NB - np.arange(NB)).astype(np.float32), (128, 1))

    SUB = C // 128
    in_maps = []
    pair_list = []       # (b, q, idx) per global pair
    for b in range(B):
        for q in range(Q):
            pair_list.append((b, q, sels[b][q]))

    for c in range(NCORES):
        m = dict(shared)
        mf_c = np.empty((PPC, 2, 128, C), np.float32)
        xyz_c = np.empty((PPC, 128, SUB, 3), np.float32)
        emt_c = np.empty((PPC, 256), np.float32)
        for p_ in range(PPC):
            b, q, idx = pair_list[c * PPC + p_]
            pad = np.zeros(C, np.int64)
            pad[:len(idx)] = idx
            mf_c[p_] = mask_feats[b][:, pad].reshape(2, 128, C)
            xyz_c[p_] = xyz[b][pad].reshape(SUB, 128, 3).transpose(1, 0, 2)
            emt_c[p_] = embedding[b, q]
        m["mf"] = mf_c
        m["xyzs"] = xyz_c
        m["emt"] = np.ascontiguousarray(emt_c.T.reshape(2, 128, PPC))
        in_maps.append(m)

    nc = _get_nc(C)
    res = run_bass_kernel_spmd(nc, in_maps, core_ids=list(range(NCORES)))
    LAST_RESULT = res

    grasps = np.zeros((B, Q, N, 4, 4), np.float32)
    for c in range(NCORES):
        g = res.results[c]["go"].reshape(PPC, C, 4, 4)
        for p_ in range(PPC):
            b, q, idx = pair_list[c * PPC + p_]
            grasps[b, q, idx] = g[p_, :len(idx)]
    mask = (confidence > thr_v).astype(np.float32)
    gconf = confidence * mask
    return grasps, gconf


# revision 15
# speedup vs baseline: 1.2103x; 1.2103x over previous
"""Trainium2 Bass kernel for the ActionDecoder problem.

Strategy
--------
1. Raggedness: grasps for points with confidence <= mask_thresh are zeroed by
   the reference, so their MLP work is skipped entirely.  The host compacts
   each (b, q) row set to the selected indices (gathering mask_feats / xyz
   columns), pads to a runtime capacity C (multiple of 512), and scatters the
   kernel's compacted output back into the zero-initialized full tensor.
   grasp_confidence = confidence * mask is pure input masking done on host.
2. Sharding: the 16 (b, q) pairs are split across 8 cores (2 pairs each);
   every core carries the full (small) MLP weights.
3. Precision: matmuls run as float32r (PE full rate, e10m11 truncation) in an
   error-free 3-term hi/lo split: x = xh + xl, W = Wh + Wl exactly at m11,
   and x@W = xh@Wh + xh@Wl + xl@Wh accumulated in one fp32 PSUM (the dropped
   xl@Wl term is ~2^-24 relative) -> fp32-grade results at 3 cycles/row.
4. Layer 1 input is concat(feat, emb):  inp @ W1 = feat @ W1[:256] +
   emb @ W1[256:].  The emb part (+ b1) is a tiny exact-fp32 matmul per
   (mlp, pair); the feat part accumulates into the same PSUM chunk and goes
   through a fused PSUM->relu(+bias) into h1, split hi/lo for layer 2.
5. Head outputs ([3],[3],[10]) are PE-transposed into a rows-on-partitions
   [128, rows, 16] layout where the epilogue (normalize, Gram-Schmidt, cross,
   argmax bin lookup, grasp assembly) runs as batched DVE/ACT ops per pair.
"""

import numpy as np

import concourse.bass as bass
import concourse.mybir as mybir
from concourse import bacc
from concourse.tile import TileContext
from concourse.bass_utils import run_bass_kernel_spmd
from concourse.masks import make_identity

B, Q, N = 2, 8, 8192
NCORES = 8
PAIRS = B * Q             # 16 (b,q) pairs, 2 per core
PPC = PAIRS // NCORES     # pairs per core
MASK = 256
HID = 512
NB = 10
T = 512                   # row-chunk (matmul moving dim)
GRIP = 0.1034

F32 = mybir.dt.float32
F32R = mybir.dt.float32r
AX = mybir.AluOpType
ACTF = mybir.ActivationFunctionType
AXL = mybir.AxisListType

HEAD_D = [3, 3, 10]       # contact, approach, offset
HEAD_OFF = [0, 3, 6]      # column offsets in the [128, rows, 16] layout

_CACHE: dict = {}

LAST_RESULT = None        # BassKernelResults of the most recent run (for profiling)


def _t22(x):
    """Truncate fp32 to fp22=e10m11 (11-bit mantissa) - matches the PE's
    float32r load, so device truncation of these values is the identity."""
    xi = np.ascontiguousarray(x, np.float32).view(np.uint32)
    return (xi & np.uint32(0xFFFFF000)).view(np.float32)


def _split22(x):
    """Exact hi/lo fp22 split of fp32 data: x == hi + lo bitwise."""
    hi = _t22(x)
    lo = np.asarray(x, np.float32) - hi
    return hi, lo


def _build(C):
    """Build the kernel for per-pair row capacity C (multiple of 512)."""
    NCH = C // T          # chunks per pair
    SUB = C // 128        # 128-row groups per pair
    M = PPC * SUB         # row-groups per core across its pairs

    nc = bacc.Bacc(None, target_bir_lowering=False, debug=False)

    # ---- per-core DRAM parameters -------------------------------------
    # compacted, per-pair tensors (leading dim = local pair index)
    mf = nc.declare_dram_parameter("mf", [PPC, 2, 128, C], F32, isOutput=False)
    emt = nc.declare_dram_parameter("emt", [2, 128, PPC], F32, isOutput=False)
    xyzs = nc.declare_dram_parameter("xyzs", [PPC, 128, SUB, 3], F32, isOutput=False)

    w1fp, w2p, w3p, w1e, b1, b2 = [], [], [], [], [], []
    for i, d in enumerate(HEAD_D):
        w1fp.append([nc.declare_dram_parameter(f"w1f{s}{i}", [2, 128, HID], F32R, isOutput=False)
                     for s in "hl"])
        w2p.append([nc.declare_dram_parameter(f"w2{s}{i}", [4, 128, HID], F32R, isOutput=False)
                    for s in "hl"])
        w3p.append([nc.declare_dram_parameter(f"w3{s}{i}", [4, 128, d], F32R, isOutput=False)
                    for s in "hl"])
        w1e.append(nc.declare_dram_parameter(f"w1e{i}", [2, 128, HID], F32, isOutput=False))
        b1.append(nc.declare_dram_parameter(f"b1{i}", [4, 128], F32, isOutput=False))
        b2.append(nc.declare_dram_parameter(f"b2{i}", [4, 128], F32, isOutput=False))
    b3row = nc.declare_dram_parameter("b3row", [128, 16], F32, isOutput=False)
    ovals = nc.declare_dram_parameter("ovals", [128, NB], F32, isOutput=False)
    desc = nc.declare_dram_parameter("desc", [128, NB], F32, isOutput=False)

    go = nc.declare_dram_parameter("go", [PPC, SUB, 128, 4, 4], F32, isOutput=True)

    with TileContext(nc) as tc:
        with (
            tc.tile_pool(name="const", bufs=1) as cpool,
            tc.tile_pool(name="wts", bufs=2) as wtpool,
            tc.tile_pool(name="io", bufs=2) as iopool,
            tc.tile_pool(name="work", bufs=2) as wpool,
            tc.tile_pool(name="epi", bufs=1) as epool,
            tc.tile_pool(name="ps", bufs=3, space="PSUM") as pspool,
            tc.tile_pool(name="pshd", bufs=2, space="PSUM") as hdpool,
        ):
            # ---- constants into SBUF -----------------------------------
            ident = cpool.tile([128, 128], F32)
            make_identity(nc, ident)

            w1e_sb, b1_sb, b2_sb, w3_sb = [], [], [], []
            for i, d in enumerate(HEAD_D):
                t2 = cpool.tile([128, 2, HID], F32, name=f"w1e_sb{i}")
                nc.sync.dma_start(t2[:], w1e[i].ap().rearrange("k p h -> p k h"))
                w1e_sb.append(t2)
                t5 = cpool.tile([128, 4], F32, name=f"b1_sb{i}")
                nc.sync.dma_start(t5[:], b1[i].ap().rearrange("m p -> p m"))
                b1_sb.append(t5)
                t6 = cpool.tile([128, 4], F32, name=f"b2_sb{i}")
                nc.sync.dma_start(t6[:], b2[i].ap().rearrange("m p -> p m"))
                b2_sb.append(t6)
                pair = []
                for s in range(2):
                    t7 = cpool.tile([128, 4, d], F32R, name=f"w3_sb{i}_{s}")
                    nc.sync.dma_start(t7[:], w3p[i][s].ap().rearrange("k p h -> p k h"))
                    pair.append(t7)
                w3_sb.append(pair)
            b3row_sb = cpool.tile([128, 16], F32)
            nc.sync.dma_start(b3row_sb[:], b3row.ap())
            ovals_sb = cpool.tile([128, NB], F32)
            nc.sync.dma_start(ovals_sb[:], ovals.ap())
            desc_sb = cpool.tile([128, NB], F32)
            nc.sync.dma_start(desc_sb[:], desc.ap())

            # ---- E1 + b1 (exact fp32 matmul, tiny): be1[i] [128, 4, PPC]
            emt_sb = cpool.tile([128, 2, PPC], F32)
            nc.sync.dma_start(emt_sb[:], emt.ap().rearrange("k p q -> p k q"))
            be1 = []
            for i in range(3):
                bt = cpool.tile([128, 4, PPC], F32, name=f"be1_{i}")
                for m in range(4):
                    ps = pspool.tile([128, T], F32, tag="mm")
                    for k in range(2):
                        nc.tensor.matmul(
                            ps[:, :PPC],
                            w1e_sb[i][:, k, m * 128:(m + 1) * 128],
                            emt_sb[:, k, :],
                            start=(k == 0), stop=(k == 1),
                        )
                    nc.vector.tensor_scalar(
                        bt[:, m, :], ps[:, :PPC], b1_sb[i][:, m:m + 1], None, AX.add,
                    )
                be1.append(bt)

            # ---- main loops --------------------------------------------
            out_rows = epool.tile([128, M, 16], F32, name="out_rows", tag="orows")
            xyz_sb = []
            for p_ in range(PPC):
                xt = iopool.tile([128, SUB, 3], F32, name=f"xyz_sb{p_}", tag="xyz", bufs=2)
                nc.sync.dma_start(xt[:], xyzs[p_])
                xyz_sb.append(xt)

            for p_ in range(PPC):
                for i in range(3):
                    d = HEAD_D[i]
                    off = HEAD_OFF[i]
                    w1f_t, w2_t = [], []
                    for s in range(2):
                        wt = wtpool.tile([128, 2, HID], F32R, name=f"w1f_t{s}",
                                         tag=f"w1f{s}", bufs=2)
                        nc.sync.dma_start(wt[:], w1fp[i][s].ap().rearrange("k p h -> p k h"))
                        w1f_t.append(wt)
                        wt2 = wtpool.tile([128, 4, HID], F32R, name=f"w2_t{s}",
                                          tag=f"w2{s}", bufs=2)
                        nc.sync.dma_start(wt2[:], w2p[i][s].ap().rearrange("k p h -> p k h"))
                        w2_t.append(wt2)

                    # 3-stage software pipeline over chunks so PE never waits
                    # on the ACT/DVE relu+split chains:
                    #   A(ch): mf load/split + L1 matmuls + relu -> h1 hi/lo
                    #   B(ch): L2 matmuls + relu -> h2 hi/lo
                    #   C(ch): L3 head + transpose into out_rows
                    st = {}

                    def stage_a(ch, p_=p_, i=i):
                        cs = slice(ch * T, (ch + 1) * T)
                        mff_sb = iopool.tile([128, 2, T], F32, name="mff_sb",
                                             tag="mff", bufs=2)
                        nc.sync.dma_start(
                            mff_sb[:], mf[p_, :, :, cs].rearrange("k p n -> p k n"))
                        mfh_sb = iopool.tile([128, 2, T], F32R, name="mfh_sb",
                                             tag="mfh", bufs=2)
                        nc.vector.tensor_copy(mfh_sb[:], mff_sb[:])
                        mfl_sb = iopool.tile([128, 2, T], F32R, name="mfl_sb",
                                             tag="mfl", bufs=2)
                        nc.vector.tensor_tensor(
                            mfl_sb[:], mff_sb[:], mfh_sb[:].bitcast(F32), AX.subtract)
                        h1f = wpool.tile([128, 4, T], F32, name="h1f", tag="h1f", bufs=1)
                        h1h = wpool.tile([128, 4, T], F32R, name="h1h", tag="h1h")
                        h1l = wpool.tile([128, 4, T], F32R, name="h1l", tag="h1l")
                        for m in range(4):
                            ms = slice(m * 128, (m + 1) * 128)
                            ps = pspool.tile([128, T], F32, tag="mm")
                            j = 0
                            for k in range(2):
                                for wa, xa in ((0, mfh_sb), (1, mfh_sb), (0, mfl_sb)):
                                    nc.tensor.matmul(
                                        ps[:], w1f_t[wa][:, k, ms],
                                        xa[:, k, :],
                                        start=(j == 0), stop=(j == 5),
                                    )
                                    j += 1
                            nc.scalar.activation(
                                h1f[:, m, :], ps[:], ACTF.Relu,
                                bias=be1[i][:, m, p_:p_ + 1],
                            )
                        nc.vector.tensor_copy(h1h[:], h1f[:])
                        nc.vector.tensor_tensor(
                            h1l[:], h1f[:], h1h[:].bitcast(F32), AX.subtract)
                        st[("h1", ch)] = (h1h, h1l)

                    def stage_b(ch, i=i):
                        h1h, h1l = st.pop(("h1", ch))
                        h2h = wpool.tile([128, 4, T], F32R, name="h2h", tag="h2h")
                        h2l = wpool.tile([128, 4, T], F32R, name="h2l", tag="h2l")
                        h2f = wpool.tile([128, 4, T], F32, name="h2f", tag="h2f", bufs=1)
                        for m in range(4):
                            ms = slice(m * 128, (m + 1) * 128)
                            ps = pspool.tile([128, T], F32, tag="mm")
                            j = 0
                            for k in range(4):
                                for wa, xa in ((0, h1h), (1, h1h), (0, h1l)):
                                    nc.tensor.matmul(
                                        ps[:], w2_t[wa][:, k, ms], xa[:, k, :],
                                        start=(j == 0), stop=(j == 11),
                                    )
                                    j += 1
                            nc.scalar.activation(
                                h2f[:, m, :], ps[:], ACTF.Relu,
                                bias=b2_sb[i][:, m:m + 1],
                            )
                        nc.vector.tensor_copy(h2h[:], h2f[:])
                        nc.vector.tensor_tensor(
                            h2l[:], h2f[:], h2h[:].bitcast(F32), AX.subtract)
                        st[("h2", ch)] = (h2h, h2l)

                    def stage_c(ch, p_=p_, i=i, d=d, off=off):
                        h2h, h2l = st.pop(("h2", ch))
                        hd = hdpool.tile([d, T], F32, tag="hd10")
                        j = 0
                        for k in range(4):
                            for wa, xa in ((0, h2h), (1, h2h), (0, h2l)):
                                nc.tensor.matmul(
                                    hd[:], w3_sb[i][wa][:, k, :], xa[:, k, :],
                                    start=(j == 0), stop=(j == 11),
                                )
                                j += 1
                        sc = wpool.tile([d, T], F32, name="sc", tag="sc10")
                        nc.vector.tensor_copy(sc[:], hd[:])
                        pt = pspool.tile([128, 4, 16], F32, tag="pt")
                        for c4 in range(T // 128):
                            nc.tensor.transpose(
                                pt[:, c4, :d],
                                sc[:, c4 * 128:(c4 + 1) * 128],
                                ident[:d, :d],
                            )
                        nc.vector.tensor_copy(
                            out_rows[:, p_ * SUB + ch * 4: p_ * SUB + ch * 4 + 4,
                                     off:off + d],
                            pt[:, :, :d],
                        )

                    stage_a(0)
                    if NCH > 1:
                        stage_a(1)
                    stage_b(0)
                    for ch in range(2, NCH):
                        stage_a(ch)
                        stage_b(ch - 1)
                        stage_c(ch - 2)
                    if NCH > 1:
                        stage_b(NCH - 1)
                        stage_c(NCH - 2)
                    stage_c(NCH - 1)

            # ---- epilogue over all rows of this core's pairs -----------
            nc.vector.tensor_tensor(
                out_rows[:], out_rows[:],
                b3row_sb[:, None, :].to_broadcast([128, M, 16]), AX.add,
            )
            c_raw = out_rows[:, :, 0:3]
            a_raw = out_rows[:, :, 3:6]
            lg = out_rows[:, :, 6:16]

            sq = epool.tile([128, M, 3], F32, name="sq", tag="sq")
            s1 = epool.tile([128, M, 1], F32, name="s1", tag="s1")
            inv = epool.tile([128, M, 1], F32, name="inv", tag="inv")
            contact = epool.tile([128, M, 3], F32, name="contact", tag="contact")
            approach = epool.tile([128, M, 3], F32, name="approach", tag="approach")
            dotp = epool.tile([128, M, 1], F32, name="dotp", tag="dotp")
            tmp3 = epool.tile([128, M, 3], F32, name="tmp3", tag="tmp3")
            cr = epool.tile([128, M, 3], F32, name="cr", tag="cr")
            tr = epool.tile([128, M, 3], F32, name="tr", tag="tr")
            t10 = epool.tile([128, M, NB], F32, name="t10", tag="t10")
            vmax = epool.tile([128, M, 1], F32, name="vmax", tag="vmax")
            offs = epool.tile([128, M, 1], F32, name="offs", tag="offs")
            g4 = epool.tile([128, M, 4, 4], F32, name="g4", tag="g4")

            def _normalize(dst, src):
                nc.vector.tensor_tensor(sq[:], src, src, AX.mult)
                nc.vector.reduce_sum(s1[:], sq[:], axis=AXL.X)
                nc.scalar.activation(s1[:], s1[:], ACTF.Sqrt)
                nc.vector.tensor_scalar_max(s1[:], s1[:], 1e-12)
                nc.vector.reciprocal(inv[:], s1[:])
                nc.vector.tensor_tensor(
                    dst, src, inv[:].to_broadcast([128, M, 3]), AX.mult)

            _normalize(contact[:], c_raw)
            # Gram-Schmidt: approach = normalize(a - contact * <a, contact>)
            nc.vector.tensor_tensor(sq[:], a_raw, contact[:], AX.mult)
            nc.vector.reduce_sum(dotp[:], sq[:], axis=AXL.X)
            nc.vector.tensor_tensor(
                tmp3[:], contact[:], dotp[:].to_broadcast([128, M, 3]), AX.mult)
            nc.vector.tensor_tensor(tmp3[:], a_raw, tmp3[:], AX.subtract)
            _normalize(approach[:], tmp3[:])

            # cross(approach, contact)
            for j in range(3):
                j1, j2 = (j + 1) % 3, (j + 2) % 3
                nc.vector.tensor_tensor(
                    cr[:, :, j:j + 1], approach[:, :, j1:j1 + 1],
                    contact[:, :, j2:j2 + 1], AX.mult)
                nc.vector.tensor_tensor(
                    tmp3[:, :, j:j + 1], approach[:, :, j2:j2 + 1],
                    contact[:, :, j1:j1 + 1], AX.mult)
            nc.vector.tensor_tensor(cr[:], cr[:], tmp3[:], AX.subtract)

            # offsets = offset_vals[argmax(lg)] (first max wins)
            nc.vector.reduce_max(vmax[:], lg, axis=AXL.X)
            nc.vector.tensor_tensor(
                t10[:], lg, vmax[:].to_broadcast([128, M, NB]), AX.is_equal)
            nc.vector.tensor_tensor(
                t10[:], t10[:], desc_sb[:, None, :].to_broadcast([128, M, NB]),
                AX.mult)
            nc.vector.reduce_max(vmax[:], t10[:], axis=AXL.X)
            nc.vector.tensor_tensor(
                t10[:], t10[:], vmax[:].to_broadcast([128, M, NB]), AX.is_equal)
            nc.vector.tensor_tensor(
                t10[:], t10[:], ovals_sb[:, None, :].to_broadcast([128, M, NB]),
                AX.mult)
            nc.vector.reduce_sum(offs[:], t10[:], axis=AXL.X)
            nc.vector.tensor_scalar_mul(offs[:], offs[:], 0.5)

            # trans = xyz + contact * offs/2 - GRIP * approach
            nc.vector.tensor_tensor(
                tr[:], contact[:], offs[:].to_broadcast([128, M, 3]), AX.mult)
            nc.vector.tensor_scalar(
                tmp3[:], approach[:], -GRIP, None, AX.mult)
            nc.vector.tensor_tensor(tr[:], tr[:], tmp3[:], AX.add)
            for p_ in range(PPC):
                nc.vector.tensor_tensor(
                    tr[:, p_ * SUB:(p_ + 1) * SUB, :],
                    tr[:, p_ * SUB:(p_ + 1) * SUB, :], xyz_sb[p_][:], AX.add)

            # grasp assembly (selected rows all have mask == 1)
            nc.vector.tensor_copy(g4[:, :, 0:3, 0], contact[:])
            nc.vector.tensor_copy(g4[:, :, 0:3, 1], cr[:])
            nc.vector.tensor_copy(g4[:, :, 0:3, 2], approach[:])
            nc.vector.tensor_copy(g4[:, :, 0:3, 3], tr[:])
            nc.vector.memset(g4[:, :, 3, 0:3], 0.0)
            nc.vector.memset(g4[:, :, 3, 3], 1.0)

            nc.sync.dma_start(
                go.ap().rearrange("q s p i j -> p (q s) i j"), g4[:])

    nc.finalize()
    return nc


def _get_nc(C):
    key = f"nc_{C}"
    if key not in _CACHE:
        _CACHE[key] = _build(C)
    return _CACHE[key]


def kernel(xyz, mask_feats, confidence, embedding, mask_thresh,
           contact_params, approach_params, offset_params, offset_vals):
    global LAST_RESULT
    xyz = np.asarray(xyz, np.float32)
    mask_feats = np.asarray(mask_feats, np.float32)
    confidence = np.asarray(confidence, np.float32)
    embedding = np.asarray(embedding, np.float32)
    thr_v = float(np.asarray(mask_thresh))
    params = [
        [(np.asarray(w, np.float32), np.asarray(bb, np.float32)) for w, bb in p]
        for p in (contact_params, approach_params, offset_params)
    ]
    offset_vals = np.asarray(offset_vals, np.float32)

    # ---- compaction: selected indices per (b, q) -----------------------
    sels = [[np.nonzero(confidence[b, q] > thr_v)[0] for q in range(Q)]
            for b in range(B)]
    max_sel = max((len(s) for bs in sels for s in bs), default=1)
    C = max(T, ((max_sel + T - 1) // T) * T)

    # ---- shared (per-core identical) tensors ---------------------------
    shared = {}
    b3cat = np.zeros(16, np.float32)
    for i, p in enumerate(params):
        w1, b1v = p[0]
        w2v, b2v = p[1]
        w3v, b3v = p[2]
        d = HEAD_D[i]
        hi, lo = _split22(w1[:MASK].reshape(2, 128, HID))
        shared[f"w1fh{i}"], shared[f"w1fl{i}"] = hi, lo
        shared[f"w1e{i}"] = np.ascontiguousarray(w1[MASK:].reshape(2, 128, HID))
        hi, lo = _split22(w2v.reshape(4, 128, HID))
        shared[f"w2h{i}"], shared[f"w2l{i}"] = hi, lo
        hi, lo = _split22(w3v.reshape(4, 128, d))
        shared[f"w3h{i}"], shared[f"w3l{i}"] = hi, lo
        shared[f"b1{i}"] = np.ascontiguousarray(b1v.reshape(4, 128))
        shared[f"b2{i}"] = np.ascontiguousarray(b2v.reshape(4, 128))
        b3cat[HEAD_OFF[i]:HEAD_OFF[i] + d] = b3v
    shared["b3row"] = np.tile(b3cat, (128, 1))
    shared["ovals"] = np.tile(offset_vals, (128, 1))
    shared["desc"] = np.tile((NB - np.arange(NB)).astype(np.float32), (128, 1))

    SUB = C // 128
    in_maps = []
    pair_list = []       # (b, q, idx) per global pair
    for b in range(B):
        for q in range(Q):
            pair_list.append((b, q, sels[b][q]))

    for c in range(NCORES):
        m = dict(shared)
        mf_c = np.empty((PPC, 2, 128, C), np.float32)
        xyz_c = np.empty((PPC, 128, SUB, 3), np.float32)
        emt_c = np.empty((PPC, 256), np.float32)
        for p_ in range(PPC):
            b, q, idx = pair_list[c * PPC + p_]
            pad = np.zeros(C, np.int64)
            pad[:len(idx)] = idx
            mf_c[p_] = mask_feats[b][:, pad].reshape(2, 128, C)
            xyz_c[p_] = xyz[b][pad].reshape(SUB, 128, 3).transpose(1, 0, 2)
            emt_c[p_] = embedding[b, q]
        m["mf"] = mf_c
        m["xyzs"] = xyz_c
        m["emt"] = np.ascontiguousarray(emt_c.T.reshape(2, 128, PPC))
        in_maps.append(m)

    nc = _get_nc(C)
    res = run_bass_kernel_spmd(nc, in_maps, core_ids=list(range(NCORES)))
    LAST_RESULT = res

    grasps = np.zeros((B, Q, N, 4, 4), np.float32)
    for c in range(NCORES):
        g = res.results[c]["go"].reshape(PPC, C, 4, 4)
        for p_ in range(PPC):
            b, q, idx = pair_list[c * PPC + p_]
            grasps[b, q, idx] = g[p_, :len(idx)]
    mask = (confidence > thr_v).astype(np.float32)
    gconf = confidence * mask
    return grasps, gconf


# revision 17
# speedup vs baseline: 1.2859x; 1.0624x over previous
"""Trainium2 Bass kernel for the ActionDecoder problem.

Strategy
--------
1. Raggedness: grasps for points with confidence <= mask_thresh are zeroed by
   the reference, so their MLP work is skipped entirely.  The host compacts
   each (b, q) row set to the selected indices (gathering mask_feats / xyz
   columns), pads to a runtime capacity C (multiple of 512), and scatters the
   kernel's compacted output back into the zero-initialized full tensor.
   grasp_confidence = confidence * mask is pure input masking done on host.
2. Sharding: the 16 (b, q) pairs are split across 8 cores (2 pairs each);
   every core carries the full (small) MLP weights.
3. Precision: matmuls run as float32r (PE full rate, e10m11 truncation) in an
   error-free 3-term hi/lo split: x = xh + xl, W = Wh + Wl exactly at m11,
   and x@W = xh@Wh + xh@Wl + xl@Wh accumulated in one fp32 PSUM (the dropped
   xl@Wl term is ~2^-24 relative) -> fp32-grade results at 3 cycles/row.
4. Layer 1 input is concat(feat, emb):  inp @ W1 = feat @ W1[:256] +
   emb @ W1[256:].  The emb part (+ b1) is a tiny exact-fp32 matmul per
   (mlp, pair); the feat part accumulates into the same PSUM chunk and goes
   through a fused PSUM->relu(+bias) into h1, split hi/lo for layer 2.
5. Head outputs ([3],[3],[10]) are PE-transposed into a rows-on-partitions
   [128, rows, 16] layout where the epilogue (normalize, Gram-Schmidt, cross,
   argmax bin lookup, grasp assembly) runs as batched DVE/ACT ops per pair.
"""

import numpy as np

import concourse.bass as bass
import concourse.mybir as mybir
from concourse import bacc
from concourse.tile import TileContext
from concourse.bass_utils import run_bass_kernel_spmd
from concourse.masks import make_identity

B, Q, N = 2, 8, 8192
NCORES = 8
PAIRS = B * Q             # 16 (b,q) pairs, 2 per core
PPC = PAIRS // NCORES     # pairs per core
MASK = 256
HID = 512
NB = 10
T = 512                   # row-chunk (matmul moving dim)
GRIP = 0.1034

F32 = mybir.dt.float32
F32R = mybir.dt.float32r
AX = mybir.AluOpType
ACTF = mybir.ActivationFunctionType
AXL = mybir.AxisListType

HEAD_D = [3, 3, 10]       # contact, approach, offset
HEAD_OFF = [0, 3, 6]      # column offsets in the [128, rows, 16] layout

_CACHE: dict = {}

LAST_RESULT = None        # BassKernelResults of the most recent run (for profiling)


def _t22(x):
    """Truncate fp32 to fp22=e10m11 (11-bit mantissa) - matches the PE's
    float32r load, so device truncation of these values is the identity."""
    xi = np.ascontiguousarray(x, np.float32).view(np.uint32)
    return (xi & np.uint32(0xFFFFF000)).view(np.float32)


def _split22(x):
    """Exact hi/lo fp22 split of fp32 data: x == hi + lo bitwise."""
    hi = _t22(x)
    lo = np.asarray(x, np.float32) - hi
    return hi, lo


def _build(C):
    """Build the kernel for per-pair row capacity C (multiple of 512)."""
    NCH = C // T          # chunks per pair
    SUB = C // 128        # 128-row groups per pair
    M = PPC * SUB         # row-groups per core across its pairs

    nc = bacc.Bacc(None, target_bir_lowering=False, debug=False)

    # ---- per-core DRAM parameters -------------------------------------
    # compacted, per-pair tensors (leading dim = local pair index)
    mf = nc.declare_dram_parameter("mf", [PPC, 2, 128, C], F32, isOutput=False)
    emt = nc.declare_dram_parameter("emt", [2, 128, PPC], F32, isOutput=False)
    xyzs = nc.declare_dram_parameter("xyzs", [PPC, 128, SUB, 3], F32, isOutput=False)

    w1fp, w2p, w3p, w1e, b1, b2 = [], [], [], [], [], []
    for i, d in enumerate(HEAD_D):
        w1fp.append([nc.declare_dram_parameter(f"w1f{s}{i}", [2, 128, HID], F32R, isOutput=False)
                     for s in "hl"])
        w2p.append([nc.declare_dram_parameter(f"w2{s}{i}", [4, 128, HID], F32R, isOutput=False)
                    for s in "hl"])
        w3p.append([nc.declare_dram_parameter(f"w3{s}{i}", [4, 128, d], F32R, isOutput=False)
                    for s in "hl"])
        w1e.append(nc.declare_dram_parameter(f"w1e{i}", [2, 128, HID], F32, isOutput=False))
        b1.append(nc.declare_dram_parameter(f"b1{i}", [4, 128], F32, isOutput=False))
        b2.append(nc.declare_dram_parameter(f"b2{i}", [4, 128], F32, isOutput=False))
    b3row = nc.declare_dram_parameter("b3row", [128, 16], F32, isOutput=False)
    ovals = nc.declare_dram_parameter("ovals", [128, NB], F32, isOutput=False)
    desc = nc.declare_dram_parameter("desc", [128, NB], F32, isOutput=False)

    go = nc.declare_dram_parameter("go", [PPC, SUB, 128, 4, 4], F32, isOutput=True)

    with TileContext(nc) as tc:
        with (
            tc.tile_pool(name="const", bufs=1) as cpool,
            tc.tile_pool(name="wts", bufs=2) as wtpool,
            tc.tile_pool(name="io", bufs=2) as iopool,
            tc.tile_pool(name="work", bufs=2) as wpool,
            tc.tile_pool(name="epi", bufs=1) as epool,
            tc.tile_pool(name="ps", bufs=3, space="PSUM") as pspool,
            tc.tile_pool(name="pshd", bufs=2, space="PSUM") as hdpool,
        ):
            # ---- constants into SBUF -----------------------------------
            ident = cpool.tile([128, 128], F32)
            make_identity(nc, ident)

            w1e_sb, b1_sb, b2_sb, w3_sb = [], [], [], []
            for i, d in enumerate(HEAD_D):
                t2 = cpool.tile([128, 2, HID], F32, name=f"w1e_sb{i}")
                nc.sync.dma_start(t2[:], w1e[i].ap().rearrange("k p h -> p k h"))
                w1e_sb.append(t2)
                t5 = cpool.tile([128, 4], F32, name=f"b1_sb{i}")
                nc.sync.dma_start(t5[:], b1[i].ap().rearrange("m p -> p m"))
                b1_sb.append(t5)
                t6 = cpool.tile([128, 4], F32, name=f"b2_sb{i}")
                nc.sync.dma_start(t6[:], b2[i].ap().rearrange("m p -> p m"))
                b2_sb.append(t6)
                pair = []
                for s in range(2):
                    t7 = cpool.tile([128, 4, d], F32R, name=f"w3_sb{i}_{s}")
                    nc.sync.dma_start(t7[:], w3p[i][s].ap().rearrange("k p h -> p k h"))
                    pair.append(t7)
                w3_sb.append(pair)
            b3row_sb = cpool.tile([128, 16], F32)
            nc.sync.dma_start(b3row_sb[:], b3row.ap())
            ovals_sb = cpool.tile([128, NB], F32)
            nc.sync.dma_start(ovals_sb[:], ovals.ap())
            desc_sb = cpool.tile([128, NB], F32)
            nc.sync.dma_start(desc_sb[:], desc.ap())

            # ---- E1 + b1 (exact fp32 matmul, tiny): be1[i] [128, 4, PPC]
            emt_sb = cpool.tile([128, 2, PPC], F32)
            nc.sync.dma_start(emt_sb[:], emt.ap().rearrange("k p q -> p k q"))
            be1 = []
            for i in range(3):
                bt = cpool.tile([128, 4, PPC], F32, name=f"be1_{i}")
                for m in range(4):
                    ps = pspool.tile([128, T], F32, tag="mm")
                    for k in range(2):
                        nc.tensor.matmul(
                            ps[:, :PPC],
                            w1e_sb[i][:, k, m * 128:(m + 1) * 128],
                            emt_sb[:, k, :],
                            start=(k == 0), stop=(k == 1),
                        )
                    nc.vector.tensor_scalar(
                        bt[:, m, :], ps[:, :PPC], b1_sb[i][:, m:m + 1], None, AX.add,
                    )
                be1.append(bt)

            # ---- main loops --------------------------------------------
            # chunk list: 512-row chunks plus an optional 256 remainder
            CHS = [T] * (C // T) + ([C % T] if C % T else [])
            CH0 = [sum(CHS[:j]) for j in range(len(CHS))]
            NCH = len(CHS)

            xyz_sb = []
            for p_ in range(PPC):
                xt = iopool.tile([128, SUB, 3], F32, name=f"xyz_sb{p_}", tag="xyz", bufs=2)
                nc.sync.dma_start(xt[:], xyzs[p_])
                xyz_sb.append(xt)

            def epilogue(p_, out_rows):
                MR = SUB  # row-groups for this pair
                nc.vector.tensor_tensor(
                    out_rows[:], out_rows[:],
                    b3row_sb[:, None, :].to_broadcast([128, MR, 16]), AX.add,
                )
                c_raw = out_rows[:, :, 0:3]
                a_raw = out_rows[:, :, 3:6]
                lg = out_rows[:, :, 6:16]

                sq = epool.tile([128, MR, 3], F32, name="sq", tag="sq")
                s1 = epool.tile([128, MR, 1], F32, name="s1", tag="s1")
                inv = epool.tile([128, MR, 1], F32, name="inv", tag="inv")
                contact = epool.tile([128, MR, 3], F32, name="contact", tag="contact")
                approach = epool.tile([128, MR, 3], F32, name="approach", tag="approach")
                dotp = epool.tile([128, MR, 1], F32, name="dotp", tag="dotp")
                tmp3 = epool.tile([128, MR, 3], F32, name="tmp3", tag="tmp3")
                cr = epool.tile([128, MR, 3], F32, name="cr", tag="cr")
                tr = epool.tile([128, MR, 3], F32, name="tr", tag="tr")
                t10 = epool.tile([128, MR, NB], F32, name="t10", tag="t10")
                vmax = epool.tile([128, MR, 1], F32, name="vmax", tag="vmax")
                offs = epool.tile([128, MR, 1], F32, name="offs", tag="offs")
                g4 = epool.tile([128, MR, 4, 4], F32, name="g4", tag="g4")

                def _normalize(dst, srcv):
                    nc.vector.tensor_tensor(sq[:], srcv, srcv, AX.mult)
                    nc.vector.reduce_sum(s1[:], sq[:], axis=AXL.X)
                    nc.scalar.activation(s1[:], s1[:], ACTF.Sqrt)
                    nc.vector.tensor_scalar_max(s1[:], s1[:], 1e-12)
                    nc.vector.reciprocal(inv[:], s1[:])
                    nc.vector.tensor_tensor(
                        dst, srcv, inv[:].to_broadcast([128, MR, 3]), AX.mult)

                _normalize(contact[:], c_raw)
                # Gram-Schmidt: approach = normalize(a - contact * <a, contact>)
                nc.vector.tensor_tensor(sq[:], a_raw, contact[:], AX.mult)
                nc.vector.reduce_sum(dotp[:], sq[:], axis=AXL.X)
                nc.vector.tensor_tensor(
                    tmp3[:], contact[:], dotp[:].to_broadcast([128, MR, 3]), AX.mult)
                nc.vector.tensor_tensor(tmp3[:], a_raw, tmp3[:], AX.subtract)
                _normalize(approach[:], tmp3[:])

                # cross(approach, contact)
                for j in range(3):
                    j1, j2 = (j + 1) % 3, (j + 2) % 3
                    nc.vector.tensor_tensor(
                        cr[:, :, j:j + 1], approach[:, :, j1:j1 + 1],
                        contact[:, :, j2:j2 + 1], AX.mult)
                    nc.vector.tensor_tensor(
                        tmp3[:, :, j:j + 1], approach[:, :, j2:j2 + 1],
                        contact[:, :, j1:j1 + 1], AX.mult)
                nc.vector.tensor_tensor(cr[:], cr[:], tmp3[:], AX.subtract)

                # offsets = offset_vals[argmax(lg)] (first max wins)
                nc.vector.reduce_max(vmax[:], lg, axis=AXL.X)
                nc.vector.tensor_tensor(
                    t10[:], lg, vmax[:].to_broadcast([128, MR, NB]), AX.is_equal)
                nc.vector.tensor_tensor(
                    t10[:], t10[:], desc_sb[:, None, :].to_broadcast([128, MR, NB]),
                    AX.mult)
                nc.vector.reduce_max(vmax[:], t10[:], axis=AXL.X)
                nc.vector.tensor_tensor(
                    t10[:], t10[:], vmax[:].to_broadcast([128, MR, NB]), AX.is_equal)
                nc.vector.tensor_tensor(
                    t10[:], t10[:], ovals_sb[:, None, :].to_broadcast([128, MR, NB]),
                    AX.mult)
                nc.vector.reduce_sum(offs[:], t10[:], axis=AXL.X)
                nc.vector.tensor_scalar_mul(offs[:], offs[:], 0.5)

                # trans = xyz + contact * offs/2 - GRIP * approach
                nc.vector.tensor_tensor(
                    tr[:], contact[:], offs[:].to_broadcast([128, MR, 3]), AX.mult)
                nc.vector.tensor_scalar(
                    tmp3[:], approach[:], -GRIP, None, AX.mult)
                nc.vector.tensor_tensor(tr[:], tr[:], tmp3[:], AX.add)
                nc.vector.tensor_tensor(tr[:], tr[:], xyz_sb[p_][:], AX.add)

                # grasp assembly (selected rows all have mask == 1)
                nc.vector.tensor_copy(g4[:, :, 0:3, 0], contact[:])
                nc.vector.tensor_copy(g4[:, :, 0:3, 1], cr[:])
                nc.vector.tensor_copy(g4[:, :, 0:3, 2], approach[:])
                nc.vector.tensor_copy(g4[:, :, 0:3, 3], tr[:])
                nc.vector.memset(g4[:, :, 3, 0:3], 0.0)
                nc.vector.memset(g4[:, :, 3, 3], 1.0)

                nc.sync.dma_start(
                    go[p_].rearrange("s p i j -> p s i j"), g4[:])

            for p_ in range(PPC):
                out_rows = epool.tile([128, SUB, 16], F32, name="out_rows",
                                      tag="orows", bufs=2)
                for i in range(3):
                    d = HEAD_D[i]
                    off = HEAD_OFF[i]
                    w1f_t, w2_t = [], []
                    for s in range(2):
                        wt = wtpool.tile([128, 2, HID], F32R, name=f"w1f_t{s}",
                                         tag=f"w1f{s}", bufs=2)
                        nc.sync.dma_start(wt[:], w1fp[i][s].ap().rearrange("k p h -> p k h"))
                        w1f_t.append(wt)
                        wt2 = wtpool.tile([128, 4, HID], F32R, name=f"w2_t{s}",
                                          tag=f"w2{s}", bufs=2)
                        nc.sync.dma_start(wt2[:], w2p[i][s].ap().rearrange("k p h -> p k h"))
                        w2_t.append(wt2)

                    # 3-stage software pipeline over chunks so PE never waits
                    # on the ACT/DVE relu+split chains:
                    #   A(ch): mf load/split + L1 matmuls + relu -> h1 hi/lo
                    #   B(ch): L2 matmuls + relu -> h2 hi/lo
                    #   C(ch): L3 head + transpose into out_rows
                    st = {}

                    def stage_a(ch, p_=p_, i=i, w1f_t=w1f_t, st=st):
                        tch = CHS[ch]
                        cs = slice(CH0[ch], CH0[ch] + tch)
                        mff_sb = iopool.tile([128, 2, T], F32, name="mff_sb",
                                             tag="mff", bufs=2)
                        nc.sync.dma_start(
                            mff_sb[:, :, :tch],
                            mf[p_, :, :, cs].rearrange("k p n -> p k n"))
                        mfh_sb = iopool.tile([128, 2, T], F32R, name="mfh_sb",
                                             tag="mfh", bufs=2)
                        nc.vector.tensor_copy(mfh_sb[:, :, :tch], mff_sb[:, :, :tch])
                        mfl_sb = iopool.tile([128, 2, T], F32R, name="mfl_sb",
                                             tag="mfl", bufs=2)
                        nc.vector.tensor_tensor(
                            mfl_sb[:, :, :tch], mff_sb[:, :, :tch],
                            mfh_sb[:, :, :tch].bitcast(F32), AX.subtract)
                        h1f = wpool.tile([128, 4, T], F32, name="h1f", tag="h1f", bufs=1)
                        h1h = wpool.tile([128, 4, T], F32R, name="h1h", tag="h1h")
                        h1l = wpool.tile([128, 4, T], F32R, name="h1l", tag="h1l")
                        for m in range(4):
                            ms = slice(m * 128, (m + 1) * 128)
                            ps = pspool.tile([128, T], F32, tag="mm")
                            j = 0
                            for k in range(2):
                                for wa, xa in ((0, mfh_sb), (1, mfh_sb), (0, mfl_sb)):
                                    nc.tensor.matmul(
                                        ps[:, :tch], w1f_t[wa][:, k, ms],
                                        xa[:, k, :tch],
                                        start=(j == 0), stop=(j == 5),
                                    )
                                    j += 1
                            nc.scalar.activation(
                                h1f[:, m, :tch], ps[:, :tch], ACTF.Relu,
                                bias=be1[i][:, m, p_:p_ + 1],
                            )
                        nc.vector.tensor_copy(h1h[:, :, :tch], h1f[:, :, :tch])
                        nc.vector.tensor_tensor(
                            h1l[:, :, :tch], h1f[:, :, :tch],
                            h1h[:, :, :tch].bitcast(F32), AX.subtract)
                        st[("h1", ch)] = (h1h, h1l)

                    def stage_b(ch, i=i, w2_t=w2_t, st=st):
                        tch = CHS[ch]
                        h1h, h1l = st.pop(("h1", ch))
                        h2h = wpool.tile([128, 4, T], F32R, name="h2h", tag="h2h")
                        h2l = wpool.tile([128, 4, T], F32R, name="h2l", tag="h2l")
                        h2f = wpool.tile([128, 4, T], F32, name="h2f", tag="h2f", bufs=1)
                        for m in range(4):
                            ms = slice(m * 128, (m + 1) * 128)
                            ps = pspool.tile([128, T], F32, tag="mm")
                            j = 0
                            for k in range(4):
                                for wa, xa in ((0, h1h), (1, h1h), (0, h1l)):
                                    nc.tensor.matmul(
                                        ps[:, :tch], w2_t[wa][:, k, ms],
                                        xa[:, k, :tch],
                                        start=(j == 0), stop=(j == 11),
                                    )
                                    j += 1
                            nc.scalar.activation(
                                h2f[:, m, :tch], ps[:, :tch], ACTF.Relu,
                                bias=b2_sb[i][:, m:m + 1],
                            )
                        nc.vector.tensor_copy(h2h[:, :, :tch], h2f[:, :, :tch])
                        nc.vector.tensor_tensor(
                            h2l[:, :, :tch], h2f[:, :, :tch],
                            h2h[:, :, :tch].bitcast(F32), AX.subtract)
                        st[("h2", ch)] = (h2h, h2l)

                    def stage_c(ch, p_=p_, i=i, d=d, off=off, st=st,
                                out_rows=out_rows):
                        tch = CHS[ch]
                        h2h, h2l = st.pop(("h2", ch))
                        hd = hdpool.tile([d, T], F32, tag="hd10")
                        j = 0
                        for k in range(4):
                            for wa, xa in ((0, h2h), (1, h2h), (0, h2l)):
                                nc.tensor.matmul(
                                    hd[:, :tch], w3_sb[i][wa][:, k, :],
                                    xa[:, k, :tch],
                                    start=(j == 0), stop=(j == 11),
                                )
                                j += 1
                        sc = wpool.tile([d, T], F32, name="sc", tag="sc10")
                        nc.vector.tensor_copy(sc[:, :tch], hd[:, :tch])
                        pt = pspool.tile([128, 4, 16], F32, tag="pt")
                        ng = tch // 128
                        for c4 in range(ng):
                            nc.tensor.transpose(
                                pt[:, c4, :d],
                                sc[:, c4 * 128:(c4 + 1) * 128],
                                ident[:d, :d],
                            )
                        g0 = CH0[ch] // 128
                        nc.vector.tensor_copy(
                            out_rows[:, g0:g0 + ng, off:off + d],
                            pt[:, :ng, :d],
                        )

                    stage_a(0)
                    if NCH > 1:
                        stage_a(1)
                    stage_b(0)
                    for ch in range(2, NCH):
                        stage_a(ch)
                        stage_b(ch - 1)
                        stage_c(ch - 2)
                    if NCH > 1:
                        stage_b(NCH - 1)
                        stage_c(NCH - 2)
                    stage_c(NCH - 1)

                epilogue(p_, out_rows)

    nc.finalize()
    return nc


def _get_nc(C):
    key = f"nc_{C}"
    if key not in _CACHE:
        _CACHE[key] = _build(C)
    return _CACHE[key]


def kernel(xyz, mask_feats, confidence, embedding, mask_thresh,
           contact_params, approach_params, offset_params, offset_vals):
    global LAST_RESULT
    xyz = np.asarray(xyz, np.float32)
    mask_feats = np.asarray(mask_feats, np.float32)
    confidence = np.asarray(confidence, np.float32)
    embedding = np.asarray(embedding, np.float32)
    thr_v = float(np.asarray(mask_thresh))
    params = [
        [(np.asarray(w, np.float32), np.asarray(bb, np.float32)) for w, bb in p]
        for p in (contact_params, approach_params, offset_params)
    ]
    offset_vals = np.asarray(offset_vals, np.float32)

    # ---- compaction: selected indices per (b, q) -----------------------
    sels = [[np.nonzero(confidence[b, q] > thr_v)[0] for q in range(Q)]
            for b in range(B)]
    max_sel = max((len(s) for bs in sels for s in bs), default=1)
    C = max(256, ((max_sel + 255) // 256) * 256)

    # ---- shared (per-core identical) tensors ---------------------------
    shared = {}
    b3cat = np.zeros(16, np.float32)
    for i, p in enumerate(params):
        w1, b1v = p[0]
        w2v, b2v = p[1]
        w3v, b3v = p[2]
        d = HEAD_D[i]
        hi, lo = _split22(w1[:MASK].reshape(2, 128, HID))
        shared[f"w1fh{i}"], shared[f"w1fl{i}"] = hi, lo
        shared[f"w1e{i}"] = np.ascontiguousarray(w1[MASK:].reshape(2, 128, HID))
        hi, lo = _split22(w2v.reshape(4, 128, HID))
        shared[f"w2h{i}"], shared[f"w2l{i}"] = hi, lo
        hi, lo = _split22(w3v.reshape(4, 128, d))
        shared[f"w3h{i}"], shared[f"w3l{i}"] = hi, lo
        shared[f"b1{i}"] = np.ascontiguousarray(b1v.reshape(4, 128))
        shared[f"b2{i}"] = np.ascontiguousarray(b2v.reshape(4, 128))
        b3cat[HEAD_OFF[i]:HEAD_OFF[i] + d] = b3v
    shared["b3row"] = np.tile(b3cat, (128, 1))
    shared["ovals"] = np.tile(offset_vals, (128, 1))
    shared["desc"] = np.tile((NB - np.arange(NB)).astype(np.float32), (128, 1))

    SUB = C // 128
    in_maps = []
    pair_list = []       # (b, q, idx) per global pair
    for b in range(B):
        for q in range(Q):
            pair_list.append((b, q, sels[b][q]))

    for c in range(NCORES):
        m = dict(shared)
        mf_c = np.empty((PPC, 2, 128, C), np.float32)
        xyz_c = np.empty((PPC, 128, SUB, 3), np.float32)
        emt_c = np.empty((PPC, 256), np.float32)
        for p_ in range(PPC):
            b, q, idx = pair_list[c * PPC + p_]
            pad = np.zeros(C, np.int64)
            pad[:len(idx)] = idx
            mf_c[p_] = mask_feats[b][:, pad].reshape(2, 128, C)
            xyz_c[p_] = xyz[b][pad].reshape(SUB, 128, 3).transpose(1, 0, 2)
            emt_c[p_] = embedding[b, q]
        m["mf"] = mf_c
        m["xyzs"] = xyz_c
        m["emt"] = np.ascontiguousarray(emt_c.T.reshape(2, 128, PPC))
        in_maps.append(m)

    nc = _get_nc(C)
    res = run_bass_kernel_spmd(nc, in_maps, core_ids=list(range(NCORES)))
    LAST_RESULT = res

    grasps = np.zeros((B, Q, N, 4, 4), np.float32)
    for c in range(NCORES):
        g = res.results[c]["go"].reshape(PPC, C, 4, 4)
        for p_ in range(PPC):
            b, q, idx = pair_list[c * PPC + p_]
            grasps[b, q, idx] = g[p_, :len(idx)]
    mask = (confidence > thr_v).astype(np.float32)
    gconf = confidence * mask
    return grasps, gconf
